# revision 9
# baseline (speedup 1.0000x reference)
"""Trainium2 Bass kernel for nn_BodyMeasurements — v3.

Pipeline per (combo, 60-degree wedge) unit, 9 units per core:
  - cross-section points (x, z, w) from plane/edge intersections, centered
    at the valid-point centroid (translation-invariant perimeter).
  - bf16 prefilter scores: projections onto the SIX wedge-boundary
    directions (multiples of 60 deg) are shared by all wedges of a slot;
    per unit the score is a single elementwise max (w pre-added). Slot-1
    wedges differ per core only by a global sign (wedges w and w+3 have
    negated boundaries), handled by a per-core +-1 input.
    Measured: true support is always top-2 per partition, margin 0.063 vs
    the first excluded candidate; bf16 noise < 0.04.
  - top-2 per partition -> 2 indirect row gathers -> svd DRAM staging in
    block-major order (row = slot*128 + partition).
  - bf16 hi/lo features transposed via one PE matmul (identity rhs);
    two [7,121]x[7,128] matmuls -> survivor projections in PSUM.
  - refine: max/max_index straight over [121, 256], one indirect gather
    of extreme points; next-point shift via a diff-matrix matmul on PE.
"""

import os
import numpy as np
import ml_dtypes

bf16 = ml_dtypes.bfloat16

B, F = 4, 20908
FPAD = 20992
NPART = 128
FPP = FPAD // NPART          # 164
PPP = FPP * 3                # 492
P = NPART * PPP              # 62976
K = 720
KW = 120
KU = 121
NEG = -60000.0              # invalid-point penalty; fp16-representable
DENSITY_OVER_6 = float(985.0 / 6.0)
NCORES = 8
NUNITS = 9
SLOTS = 2
NSLOT = NPART * SLOTS        # 256
C60 = 0.5
S60 = float(np.sqrt(3.0) / 2.0)


def _sharding():
    table = []
    for c in range(NCORES):
        units = [(c, wdg, 0) for wdg in range(6)]
        ci1 = 8 + c // 2
        w0 = 3 * (c % 2)
        units += [(ci1, wdg, 1) for wdg in range(w0, w0 + 3)]
        table.append(units)
    return table


SHARD = _sharding()


def _dirs_tables():
    theta = np.arange(K, dtype=np.float32) * np.float32(2.0 * np.pi / K)
    return np.cos(theta).astype(np.float32), np.sin(theta).astype(np.float32)


def make_core_inputs(triangles, faces, bcs):
    tris9 = np.ascontiguousarray(triangles.reshape(B, F, 9).astype(np.float32))
    tris_pad = np.zeros((B, FPAD, 9), np.float32)
    tris_pad[:, :F, :] = tris9
    tris_part = tris_pad.reshape(B, NPART, FPP * 9)

    dc, ds = _dirs_tables()
    meas_faces = [faces["chest"], faces["belly"], faces["hips"]]
    meas_bcs = [bcs["chest"], bcs["belly"], bcs["hips"]]

    ins = []
    for c in range(NCORES):
        units = SHARD[c]
        b0 = units[0][0] % 4
        b1 = units[6][0] % 4
        tris_sel = np.stack([tris_part[b0], tris_part[b1]])

        # hh2: per-partition (h_slot0, h_slot1, slot1_sign)
        hh2 = np.zeros((NPART, 3), np.float32)
        for s, uu in ((0, units[0]), (1, units[6])):
            ci = uu[0]
            m, b = ci // 4, ci % 4
            ys = triangles[b, meas_faces[m], :, 1].astype(np.float32)
            bc = np.asarray(meas_bcs[m], np.float32)
            hh2[:, s] = np.float32((ys * bc).sum(dtype=np.float32))
        hh2[:, 2] = 1.0 if units[6][1] == 0 else -1.0

        # heights: sum(hgt[:, 0:6] * hgt[:, 6:12]) = head_y - heel_y
        hgt = np.zeros((4, 12), np.float32)
        for b in range(4):
            hgt[b, 0:3] = triangles[b, faces["head"], :, 1]
            hgt[b, 3:6] = triangles[b, faces["heel"], :, 1]
            hgt[b, 6:9] = np.asarray(bcs["head"], np.float32)
            hgt[b, 9:12] = -np.asarray(bcs["heel"], np.float32)

        d7all = np.zeros((3, NUNITS * KU), np.float16)
        for iu, (ci, wdg, s) in enumerate(units):
            ks = np.arange(wdg * KW, wdg * KW + KU) % K
            d7all[:, iu * KU:(iu + 1) * KU] = np.stack(
                [dc[ks].astype(np.float16), ds[ks].astype(np.float16),
                 np.ones(KU, np.float16)])

        ins.append({
            "tris": tris_sel,
            "hh2": hh2,
            "hgt": hgt,
            "d7all": d7all,
        })
    return ins


_NC_CACHE = {}


def build_kernel():
    _key = "nc" + os.environ.get("K2_REPEAT", "1")
    if _key in _NC_CACHE:
        return _NC_CACHE[_key]
    import concourse.bacc as bacc
    import concourse.bass as bass
    import concourse.mybir as mybir
    from concourse.tile import TileContext, add_dep_helper

    dt = mybir.dt
    Alu = mybir.AluOpType
    Act = mybir.ActivationFunctionType
    AX = mybir.AxisListType.X
    IOff = bass.IndirectOffsetOnAxis

    nc = bacc.Bacc("TRN2", target_bir_lowering=False, debug=False,
                   num_devices=NCORES, num_swdge_queues=4)

    tris_d = nc.dram_tensor("tris", [2, NPART, FPP * 9], dt.float32,
                            kind="ExternalInput")
    hh2_d = nc.dram_tensor("hh2", [NPART, 3], dt.float32,
                           kind="ExternalInput")
    hgt_d = nc.dram_tensor("hgt", [4, 12], dt.float32, kind="ExternalInput")
    d7all_d = nc.dram_tensor("d7all", [3, NUNITS * KU], dt.float16,
                             kind="ExternalInput")
    out_d = nc.dram_tensor("out", [16], dt.float32, kind="ExternalOutput")

    with TileContext(nc) as tc:
        with (
            tc.tile_pool(name="const", bufs=1) as cp,
            tc.tile_pool(name="rep", bufs=2) as rpp,
            tc.tile_pool(name="slotbuf", bufs=3) as sp,
            tc.tile_pool(name="proj", bufs=3) as pp,
            tc.tile_pool(name="unit", bufs=4) as upl,
            tc.tile_pool(name="utail", bufs=6) as utp,
            tc.tile_pool(name="ref", bufs=6) as rp,
            tc.tile_pool(name="pst", bufs=1, space="PSUM") as pst,
            tc.tile_pool(name="psb", bufs=2, space="PSUM") as psb,
            tc.tile_pool(name="psn", bufs=2, space="PSUM") as psnp,
            tc.tile_pool(name="pss", bufs=2, space="PSUM") as pss,
            tc.tile_pool(name="dram", bufs=1, space="DRAM") as dmp,
            tc.tile_pool(name="dramu", bufs=1, space="DRAM") as dmu,
        ):
            ones = cp.tile([NPART, 1], dt.float32, tag="ones")
            nc.vector.memset(ones[:, :], 1.0)
            epseg = cp.tile([NPART, 1], dt.float32, tag="epseg")
            nc.vector.memset(epseg[:, :], 1e-20)
            outsb = cp.tile([1, 16], dt.float32, tag="outsb")
            nc.vector.memset(outsb[:, :], 0.0)
            pbase_u = cp.tile([NPART, 1], dt.uint32, tag="pbase_u")
            nc.gpsimd.iota(pbase_u[:, :], pattern=[[0, 1]], base=0,
                           channel_multiplier=PPP)
            pbase = cp.tile([NPART, 1], dt.float32, tag="pbase")
            nc.vector.tensor_copy(pbase[:, :], pbase_u[:, :])

            # identity (fp16) for the PE transpose
            onesb = cp.tile([NPART, NPART], dt.float16, tag="onesb")
            nc.vector.memset(onesb[:, :], 1.0)
            ident = cp.tile([NPART, NPART], dt.float16, tag="ident")
            nc.gpsimd.affine_select(ident[:, :], onesb[:, :],
                                    pattern=[[-1, NPART]], base=0,
                                    channel_multiplier=1,
                                    compare_op=Alu.is_equal, fill=0.0)
            # diff matrix D[p,i] = delta(p,i) - delta(p,i+1) (fp32)
            onesf = cp.tile([KU, KU], dt.float32, tag="onesf")
            nc.vector.memset(onesf[:, :], 1.0)
            dm1 = cp.tile([KU, KU], dt.float32, tag="dm1")
            nc.gpsimd.affine_select(dm1[:, :], onesf[:, :],
                                    pattern=[[-1, KU]], base=0,
                                    channel_multiplier=1,
                                    compare_op=Alu.is_equal, fill=0.0)
            dm2 = cp.tile([KU, KU], dt.float32, tag="dm2")
            nc.gpsimd.affine_select(dm2[:, :], onesf[:, :],
                                    pattern=[[-1, KU]], base=-1,
                                    channel_multiplier=1,
                                    compare_op=Alu.is_equal, fill=0.0)
            dmat = cp.tile([KU, KU], dt.float32, tag="dmat")
            nc.vector.tensor_sub(dmat[:, :], dm1[:, :], dm2[:, :])

            d7all = cp.tile([3, NUNITS * KU], dt.float16, tag="d7all")
            nc.sync.dma_start(d7all[:, :], d7all_d[:, :])

            scr = [dmp.tile([P, 3], dt.float32, tag=f"scr{s}{par}",
                            name=f"scr{s}{par}")
                   for s in range(2) for par in range(2)]
            prev_scr_readers = {}
            prev_svd_readers = {}
            _REPEAT = int(os.environ.get('K2_REPEAT', '1'))
            for _rep in range(_REPEAT):
                _par = _rep % 2
                hh2 = rpp.tile([NPART, 3], dt.float32, tag="hh2")
                nc.sync.dma_start(hh2[:, :], hh2_d[:, :])

                # ---- heights ----
                hgt = rpp.tile([4, 12], dt.float32, tag="hgt")
                nc.sync.dma_start(hgt[:, :], hgt_d[:, :])
                hp = rpp.tile([4, 6], dt.float32, tag="hp")
                nc.any.tensor_mul(hp[:, :], hgt[:, 0:6], hgt[:, 6:12])
                hs = rpp.tile([4, 1], dt.float32, tag="hs")
                nc.vector.reduce_sum(hs[:, :], hp[:, :], axis=AX)
                habs = rpp.tile([4, 1], dt.float32, tag="habs")
                nc.scalar.activation(habs[:, :], hs[:, :], Act.Abs)
                nc.sync.dma_start(out_d[11:15], habs[:, 0:1])

                for s in range(2):
                    slot_units = (list(range(6)) if s == 0
                                  else list(range(6, 9)))
                    T = sp.tile([NPART, FPP * 9], dt.float32, tag="tris")
                    tchunk = FPP * 9 // 6
                    for ch in range(6):
                        nc.sync.dma_start(
                            T[:, ch * tchunk:(ch + 1) * tchunk],
                            tris_d[s, :, ch * tchunk:(ch + 1) * tchunk])
                    Tv = T[:, :].rearrange("p (f n) -> p f n", n=9)

                    def cview(vtx, coord):
                        return Tv[:, :, 3 * vtx + coord]

                    # ---- mass: vol = z0*m12 + z1*m20 + z2*m01 ----
                    ta = sp.tile([NPART, FPP], dt.float32, tag="ta")
                    tb = sp.tile([NPART, FPP], dt.float32, tag="tb")
                    m12 = sp.tile([NPART, FPP], dt.float32, tag="m12")
                    m20 = sp.tile([NPART, FPP], dt.float32, tag="m20")
                    m01 = sp.tile([NPART, FPP], dt.float32, tag="m01")
                    for mt, (va, vb) in ((m12, (1, 2)), (m20, (2, 0)),
                                         (m01, (0, 1))):
                        nc.gpsimd.tensor_mul(ta[:, :], cview(va, 0),
                                             cview(vb, 1))
                        nc.gpsimd.tensor_mul(tb[:, :], cview(vb, 0),
                                             cview(va, 1))
                        nc.gpsimd.tensor_sub(mt[:, :], ta[:, :], tb[:, :])
                    macc = sp.tile([NPART, FPP], dt.float32, tag="macc")
                    mac2 = sp.tile([NPART, FPP], dt.float32, tag="mac2")
                    nc.gpsimd.tensor_mul(macc[:, :], cview(0, 2), m12[:, :])
                    nc.gpsimd.tensor_mul(mac2[:, :], cview(1, 2), m20[:, :])
                    nc.any.tensor_add(macc[:, :], macc[:, :], mac2[:, :])
                    nc.gpsimd.tensor_mul(mac2[:, :], cview(2, 2), m01[:, :])
                    nc.any.tensor_add(macc[:, :], macc[:, :], mac2[:, :])
                    msum = sp.tile([NPART, 1], dt.float32, tag="msum")
                    nc.vector.reduce_sum(msum[:, :], macc[:, :], axis=AX)
                    psm = pss.tile([1, 4], dt.float32, tag="small")
                    nc.tensor.matmul(psm[0:1, 0:1], lhsT=msum[:, :],
                                     rhs=ones[:, :], start=True, stop=True)
                    nc.scalar.activation(outsb[0:1, 9 + s:10 + s],
                                         psm[0:1, 0:1], Act.Abs,
                                         scale=DENSITY_OVER_6)

                    # ---- cross-section points (x, z, w) ----
                    pts3 = sp.tile([NPART, PPP * 3], dt.float32, tag="pts3")
                    p3v = pts3[:, :].rearrange("p (f e c) -> p f e c",
                                               e=3, c=3)
                    vmsk = sp.tile([NPART, PPP], dt.float32, tag="vmsk")
                    vv = vmsk[:, :].rearrange("p (f e) -> p f e", e=3)
                    hb = hh2[:, s:s + 1]
                    # u_v = h - y_v shared by both edges touching vertex v
                    uvt = []
                    for v in range(3):
                        uv = sp.tile([NPART, FPP], dt.float32, tag=f"uv{v}")
                        nc.scalar.activation(uv[:, :], cview(v, 1),
                                             Act.Identity, bias=hb,
                                             scale=-1.0)
                        uvt.append(uv)
                    for e in range(3):
                        i, j = e, (e + 1) % 3
                        yi = cview(i, 1); yj = cview(j, 1)
                        xi = cview(i, 0); xj = cview(j, 0)
                        zi = cview(i, 2); zj = cview(j, 2)
                        tnum = uvt[i]
                        tnum2 = uvt[j]
                        dd = sp.tile([NPART, FPP], dt.float32, tag="dd")
                        nc.gpsimd.tensor_sub(dd[:, :], yj, yi)
                        rec = sp.tile([NPART, FPP], dt.float32, tag="rec")
                        nc.vector.reciprocal(rec[:, :], dd[:, :])
                        trw = sp.tile([NPART, FPP], dt.float32, tag="trw")
                        nc.any.tensor_mul(trw[:, :], tnum[:, :], rec[:, :])
                        tcl = sp.tile([NPART, FPP], dt.float32, tag="tcl")
                        nc.any.tensor_scalar(tcl[:, :], trw[:, :], 0.0,
                                             1.0, op0=Alu.max, op1=Alu.min)
                        prod = sp.tile([NPART, FPP], dt.float32,
                                       tag="prodv")
                        nc.gpsimd.tensor_mul(prod[:, :], tnum[:, :],
                                             tnum2[:, :])
                        nc.any.tensor_scalar(vv[:, :, e], prod[:, :], 0.0,
                                             None, op0=Alu.is_lt)
                        nc.any.tensor_scalar(p3v[:, :, e, 2], vv[:, :, e],
                                             -NEG, NEG, op0=Alu.mult,
                                             op1=Alu.add)
                        dxt = sp.tile([NPART, FPP], dt.float32, tag="dxt")
                        nc.gpsimd.tensor_sub(dxt[:, :], xj, xi)
                        pxm = sp.tile([NPART, FPP], dt.float32, tag="pxm")
                        nc.gpsimd.tensor_mul(pxm[:, :], tcl[:, :], dxt[:, :])
                        nc.gpsimd.tensor_add(p3v[:, :, e, 0], pxm[:, :], xi)
                        dzt = sp.tile([NPART, FPP], dt.float32, tag="dzt")
                        nc.gpsimd.tensor_sub(dzt[:, :], zj, zi)
                        pzm = sp.tile([NPART, FPP], dt.float32, tag="pzm")
                        nc.gpsimd.tensor_mul(pzm[:, :], tcl[:, :], dzt[:, :])
                        nc.gpsimd.tensor_add(p3v[:, :, e, 1], pzm[:, :], zi)

                    pall = pts3[:, :].rearrange("p (n c) -> p n c", c=3)
                    xs = pall[:, :, 0]; zs = pall[:, :, 1]

                    # ---- centroid of valid points; center in place ----
                    cxt = sp.tile([NPART, PPP], dt.float32, tag="cxt")
                    s3 = sp.tile([NPART, 3], dt.float32, tag="s3")
                    nc.gpsimd.tensor_mul(cxt[:, :], xs, vmsk[:, :])
                    nc.vector.reduce_sum(s3[:, 0:1], cxt[:, :], axis=AX)
                    nc.gpsimd.tensor_mul(cxt[:, :], zs, vmsk[:, :])
                    nc.vector.reduce_sum(s3[:, 1:2], cxt[:, :], axis=AX)
                    cxt2 = sp.tile([NPART, PPP], dt.float32, tag="cxt2")
                    nc.scalar.activation(cxt2[:, :], vmsk[:, :],
                                         Act.Identity,
                                         accum_out=s3[:, 2:3])
                    ps3 = pss.tile([1, 4], dt.float32, tag="small")
                    nc.tensor.matmul(ps3[0:1, 0:3], lhsT=ones[:, :],
                                     rhs=s3[:, :], start=True, stop=True)
                    csum = sp.tile([1, 3], dt.float32, tag="csum")
                    nc.scalar.copy(csum[:, :], ps3[0:1, 0:3])
                    cneg = sp.tile([1, 1], dt.float32, tag="cneg")
                    nc.vector.tensor_scalar_mul(cneg[:, :], csum[0:1, 2:3],
                                                -1.0)
                    crec = sp.tile([1, 1], dt.float32, tag="crec")
                    nc.vector.reciprocal(crec[:, :], cneg[0:1, 0:1])
                    cxy = sp.tile([1, 2], dt.float32, tag="cxy")
                    nc.vector.tensor_scalar_mul(cxy[:, :], csum[0:1, 0:2],
                                                crec[0:1, 0:1])
                    cb = sp.tile([NPART, 2], dt.float32, tag="cb")
                    nc.gpsimd.partition_broadcast(cb[:, :], cxy[0:1, :],
                                                  NPART)
                    nc.gpsimd.tensor_scalar(xs, xs, cb[:, 0:1], None,
                                            op0=Alu.add)
                    nc.gpsimd.tensor_scalar(zs, zs, cb[:, 1:2], None,
                                            op0=Alu.add)

                    # centered fp32 copy to DRAM scratch (parity-buffered)
                    scr_s = scr[s * 2 + _par]
                    scr_view = scr_s[:, :].rearrange("(q n) c -> q (n c)",
                                                     q=NPART)
                    schunk = PPP * 3 // 6
                    w_scrs = []
                    for ch in range(6):
                        w = nc.sync.dma_start(
                            scr_view[:, ch * schunk:(ch + 1) * schunk],
                            pts3[:, ch * schunk:(ch + 1) * schunk])
                        for g in prev_scr_readers.get((s, _par), []):
                            add_dep_helper(w.ins, g,
                                           reason="scr WAR across reps")
                        w_scrs.append(w)
                    prev_scr_readers[(s, _par)] = []

                    # ---- bf16 copies (packed) for scoring ----
                    xb = pp.tile([NPART, PPP], dt.bfloat16, tag="xb")
                    nc.scalar.copy(xb[:, :], xs)
                    zb = pp.tile([NPART, PPP], dt.bfloat16, tag="zb")
                    nc.scalar.copy(zb[:, :], zs)
                    wbv = pp.tile([NPART, PPP], dt.bfloat16, tag="wbv")
                    nc.vector.tensor_copy(wbv[:, :], pall[:, :, 2])
                    if s == 1:
                        sgn = hh2[:, 2:3]
                        xq = pp.tile([NPART, PPP], dt.bfloat16, tag="xq")
                        nc.vector.tensor_scalar_mul(xq[:, :], xb[:, :], sgn)
                        zq = pp.tile([NPART, PPP], dt.bfloat16, tag="zq")
                        nc.vector.tensor_scalar_mul(zq[:, :], zb[:, :], sgn)
                        xb, zb = xq, zq

                    # shared boundary projections, w pre-added:
                    # a = x/2, b = z*s60; p60 = a+b, p120 = b-a
                    pa_ = pp.tile([NPART, PPP], dt.bfloat16, tag="pa_")
                    nc.vector.tensor_scalar_mul(pa_[:, :], xb[:, :], C60)
                    pbt = pp.tile([NPART, PPP], dt.bfloat16, tag="pbt")
                    nc.vector.tensor_scalar_mul(pbt[:, :], zb[:, :], S60)
                    p60 = pp.tile([NPART, PPP], dt.bfloat16, tag="p60")
                    nc.any.tensor_add(p60[:, :], pa_[:, :], pbt[:, :])
                    p120 = pp.tile([NPART, PPP], dt.bfloat16, tag="p120")
                    nc.any.tensor_sub(p120[:, :], pbt[:, :], pa_[:, :])
                    p0w = pp.tile([NPART, PPP], dt.bfloat16, tag="p0w")
                    nc.any.tensor_add(p0w[:, :], xb[:, :], wbv[:, :])
                    p60w = pp.tile([NPART, PPP], dt.bfloat16, tag="p60w")
                    nc.any.tensor_add(p60w[:, :], p60[:, :], wbv[:, :])
                    p120w = pp.tile([NPART, PPP], dt.bfloat16, tag="p120w")
                    nc.any.tensor_add(p120w[:, :], p120[:, :], wbv[:, :])
                    n0w = pp.tile([NPART, PPP], dt.bfloat16, tag="n0w")
                    nc.any.tensor_sub(n0w[:, :], wbv[:, :], xb[:, :])
                    if s == 0:
                        n60w = pp.tile([NPART, PPP], dt.bfloat16,
                                       tag="n60w")
                        nc.any.tensor_sub(n60w[:, :], wbv[:, :], p60[:, :])
                        n120w = pp.tile([NPART, PPP], dt.bfloat16,
                                        tag="n120w")
                        nc.any.tensor_sub(n120w[:, :], wbv[:, :],
                                          p120[:, :])
                        wpair = {0: (p0w, p60w), 1: (p60w, p120w),
                                 2: (p120w, n0w), 3: (n0w, n60w),
                                 4: (n60w, n120w), 5: (n120w, p0w)}
                    else:
                        wpair = {0: (p0w, p60w), 1: (p60w, p120w),
                                 2: (p120w, n0w)}

                    for u in slot_units:
                        wrel = u - 6 if s == 1 else u
                        pa, pb = wpair[wrel]
                        score = upl.tile([NPART, PPP], dt.bfloat16,
                                         tag="score")
                        nc.any.tensor_max(score[:, :], pa[:, :], pb[:, :])

                        # ---- top-2 per partition ----
                        mx8 = utp.tile([NPART, 8], dt.bfloat16, tag="mx8")
                        nc.vector.max(mx8[:, :], score[:, :])
                        i8 = utp.tile([NPART, 8], dt.uint16, tag="i8")
                        nc.vector.max_index(i8[:, :], mx8[:, :],
                                            score[:, :])
                        jf = utp.tile([NPART, SLOTS], dt.float32, tag="jf")
                        nc.vector.tensor_copy(jf[:, :], i8[:, 0:SLOTS])
                        gf = utp.tile([NPART, SLOTS], dt.float32, tag="gf")
                        nc.vector.tensor_scalar(gf[:, :], jf[:, :],
                                                pbase[:, 0:1], None,
                                                op0=Alu.add)
                        offs_u = utp.tile([NPART, SLOTS], dt.uint32,
                                          tag="offs_u")
                        nc.vector.tensor_copy(offs_u[:, :], gf[:, :])

                        # ---- survivor gathers ----
                        sg = utp.tile([NPART, SLOTS * 3], dt.float32,
                                      tag="sg")
                        for jslot in range(SLOTS):
                            g_sg = nc.gpsimd.indirect_dma_start(
                                out=sg[:, jslot * 3:jslot * 3 + 3],
                                out_offset=None,
                                in_=scr_s[:, :],
                                in_offset=IOff(
                                    ap=offs_u[:, jslot:jslot + 1], axis=0))
                            for w in w_scrs:
                                add_dep_helper(g_sg.ins, w.ins,
                                               reason="scr RAW")
                            prev_scr_readers[(s, _par)].append(g_sg.ins)

                        # block-major svd: row = slot*128 + partition
                        svd = dmu.tile([NSLOT, 3], dt.float32,
                                       tag=f"svd{u}{_par}",
                                       name=f"svd{u}{_par}")
                        w_svd = nc.sync.dma_start(
                            svd[:, :].rearrange("(n q) c -> q n c",
                                                q=NPART),
                            sg[:, :].rearrange("p (n c) -> p n c", c=3))
                        for g in prev_svd_readers.get((u, _par), []):
                            add_dep_helper(w_svd.ins, g,
                                           reason="svd WAR across reps")
                        prev_svd_readers[(u, _par)] = []

                        # ---- fp16 features [128, (slot,3)] ----
                        feat = utp.tile([NPART, SLOTS * 3], dt.float16,
                                        tag="feat")
                        nc.vector.tensor_copy(feat[:, :], sg[:, :])

                        # ---- PE transpose -> rhs [3, 256] fp16 ----
                        rhsb = utp.tile([3, NSLOT], dt.float16,
                                        tag="rhsb")
                        for sl in range(SLOTS):
                            psT = pst.tile([3, NPART], dt.float16,
                                           tag=f"psT{sl}")
                            nc.tensor.matmul(psT[:, :],
                                             lhsT=feat[:, sl * 3:sl * 3 + 3],
                                             rhs=ident[:, :],
                                             is_transpose=True,
                                             start=True, stop=True)
                            nc.vector.tensor_copy(
                                rhsb[:, sl * NPART:(sl + 1) * NPART],
                                psT[:, :])

                        # ---- survivor projections [121, 256] ----
                        ps2 = psb.tile([KU, NSLOT], dt.float32, tag="ps2")
                        for sl in range(SLOTS):
                            nc.tensor.matmul(
                                ps2[:, sl * NPART:(sl + 1) * NPART],
                                lhsT=d7all[:, u * KU:(u + 1) * KU],
                                rhs=rhsb[:, sl * NPART:(sl + 1) * NPART],
                                start=True, stop=True)

                        # ---- argmax over survivors ----
                        p8m = rp.tile([KU, 8], dt.float32, tag="p8m")
                        nc.vector.max(p8m[:, :], ps2[:, :])
                        i8b = rp.tile([KU, 8], dt.uint32, tag="i8b")
                        nc.vector.max_index(i8b[:, :], p8m[:, :],
                                            ps2[:, :])

                        # ---- extreme points; segments via diff matmul ----
                        ext = rp.tile([KU, 3], dt.float32, tag="ext")
                        g_ext = nc.gpsimd.indirect_dma_start(
                            out=ext[:, :], out_offset=None,
                            in_=svd[:, :],
                            in_offset=IOff(ap=i8b[:, 0:1], axis=0))
                        add_dep_helper(g_ext.ins, w_svd.ins,
                                       reason="svd RAW")
                        prev_svd_readers[(u, _par)].append(g_ext.ins)

                        psn = psnp.tile([KU, 2], dt.float32, tag="psn")
                        nc.tensor.matmul(psn[:, :], lhsT=dmat[:, :],
                                         rhs=ext[:, 0:2], start=True,
                                         stop=True)
                        sq = rp.tile([KW, 2], dt.float32, tag="sq")
                        nc.scalar.activation(sq[:, :], psn[0:KW, :],
                                             Act.Square)
                        ssum = rp.tile([KW, 1], dt.float32, tag="ssum")
                        nc.vector.reduce_sum(ssum[:, :], sq[:, :], axis=AX)
                        segl = rp.tile([KW, 1], dt.float32, tag="segl")
                        nc.scalar.activation(segl[:, :], ssum[:, :],
                                             Act.Sqrt,
                                             bias=epseg[0:KW, 0:1])
                        psq = pss.tile([1, 4], dt.float32, tag="small")
                        nc.tensor.matmul(psq[0:1, 0:1], lhsT=segl[:, :],
                                         rhs=ones[0:KW, :], start=True,
                                         stop=True)
                        nc.scalar.copy(outsb[0:1, u:u + 1],
                                       psq[0:1, 0:1])

                nc.sync.dma_start(out_d[0:11], outsb[0:1, 0:11])

    nc.compile()
    _NC_CACHE[_key] = nc
    return nc


def assemble(core_outs):
    perim = np.zeros(12, np.float64)
    for c in range(NCORES):
        for iu, (ci, wdg, s) in enumerate(SHARD[c]):
            perim[ci] += float(core_outs[c][iu])
    mass = np.array([core_outs[b][9] for b in range(4)], np.float32)
    height = np.asarray(core_outs[0][11:15], np.float32)
    out = np.stack([
        mass, height,
        perim[0:4].astype(np.float32),
        perim[4:8].astype(np.float32),
        perim[8:12].astype(np.float32),
    ])
    return out.astype(np.float32)


def kernel(triangles, head_top_bc, left_heel_bc, chest_bcs, belly_bcs,
           hips_bcs, head_top_face_idx, left_heel_face_idx,
           chest_face_index, belly_face_index, hips_face_index):
    from concourse import bass_utils

    faces = {"head": int(head_top_face_idx), "heel": int(left_heel_face_idx),
             "chest": int(chest_face_index), "belly": int(belly_face_index),
             "hips": int(hips_face_index)}
    bcs = {"head": np.asarray(head_top_bc, np.float32),
           "heel": np.asarray(left_heel_bc, np.float32),
           "chest": np.asarray(chest_bcs, np.float32),
           "belly": np.asarray(belly_bcs, np.float32),
           "hips": np.asarray(hips_bcs, np.float32)}
    tris = np.asarray(triangles, np.float32)

    ins = make_core_inputs(tris, faces, bcs)
    nc = build_kernel()
    res = bass_utils.run_bass_kernel_spmd(nc, ins,
                                          core_ids=list(range(NCORES)))
    return assemble([r["out"] for r in res.results])


# revision 10
# speedup vs baseline: 1.5568x; 1.5568x over previous
"""Trainium2 Bass kernel for nn_BodyMeasurements — v3.

Pipeline per (combo, 60-degree wedge) unit, 9 units per core:
  - cross-section points (x, z, w) from plane/edge intersections, centered
    at the valid-point centroid (translation-invariant perimeter).
  - bf16 prefilter scores: projections onto the SIX wedge-boundary
    directions (multiples of 60 deg) are shared by all wedges of a slot;
    per unit the score is a single elementwise max (w pre-added). Slot-1
    wedges differ per core only by a global sign (wedges w and w+3 have
    negated boundaries), handled by a per-core +-1 input.
    Measured: true support is always top-2 per partition, margin 0.063 vs
    the first excluded candidate; bf16 noise < 0.04.
  - top-2 per partition -> 2 indirect row gathers -> svd DRAM staging in
    block-major order (row = slot*128 + partition).
  - bf16 hi/lo features transposed via one PE matmul (identity rhs);
    two [7,121]x[7,128] matmuls -> survivor projections in PSUM.
  - refine: max/max_index straight over [121, 256], one indirect gather
    of extreme points; next-point shift via a diff-matrix matmul on PE.
"""

import os
import numpy as np
import ml_dtypes

bf16 = ml_dtypes.bfloat16

B, F = 4, 20908
FPAD = 20992
NPART = 128
FPP = FPAD // NPART          # 164
PPP = FPP * 3                # 492
P = NPART * PPP              # 62976
K = 720
KW = 120
KU = 121
NEG = -60000.0              # invalid-point penalty; fp16-representable
DENSITY_OVER_6 = float(985.0 / 6.0)
NCORES = 8
NUNITS = 9
SLOTS = 2
NSLOT = NPART * SLOTS        # 256
C60 = 0.5
S60 = float(np.sqrt(3.0) / 2.0)


def _sharding():
    table = []
    for c in range(NCORES):
        units = [(c, wdg, 0) for wdg in range(6)]
        ci1 = 8 + c // 2
        w0 = 3 * (c % 2)
        units += [(ci1, wdg, 1) for wdg in range(w0, w0 + 3)]
        table.append(units)
    return table


SHARD = _sharding()


def _dirs_tables():
    theta = np.arange(K, dtype=np.float32) * np.float32(2.0 * np.pi / K)
    return np.cos(theta).astype(np.float32), np.sin(theta).astype(np.float32)


def make_core_inputs(triangles, faces, bcs):
    tris9 = np.ascontiguousarray(triangles.reshape(B, F, 9).astype(np.float32))
    tris_pad = np.zeros((B, FPAD, 9), np.float32)
    tris_pad[:, :F, :] = tris9
    tris_part = tris_pad.reshape(B, NPART, FPP * 9)

    dc, ds = _dirs_tables()
    meas_faces = [faces["chest"], faces["belly"], faces["hips"]]
    meas_bcs = [bcs["chest"], bcs["belly"], bcs["hips"]]

    ins = []
    for c in range(NCORES):
        units = SHARD[c]
        b0 = units[0][0] % 4
        b1 = units[6][0] % 4
        tris_sel = np.stack([tris_part[b0], tris_part[b1]])

        # hh2: per-partition (h_slot0, h_slot1, slot1_sign)
        hh2 = np.zeros((NPART, 3), np.float32)
        for s, uu in ((0, units[0]), (1, units[6])):
            ci = uu[0]
            m, b = ci // 4, ci % 4
            ys = triangles[b, meas_faces[m], :, 1].astype(np.float32)
            bc = np.asarray(meas_bcs[m], np.float32)
            hh2[:, s] = np.float32((ys * bc).sum(dtype=np.float32))
        hh2[:, 2] = 1.0 if units[6][1] == 0 else -1.0

        # heights: sum(hgt[:, 0:6] * hgt[:, 6:12]) = head_y - heel_y
        hgt = np.zeros((4, 12), np.float32)
        for b in range(4):
            hgt[b, 0:3] = triangles[b, faces["head"], :, 1]
            hgt[b, 3:6] = triangles[b, faces["heel"], :, 1]
            hgt[b, 6:9] = np.asarray(bcs["head"], np.float32)
            hgt[b, 9:12] = -np.asarray(bcs["heel"], np.float32)

        d7all = np.zeros((3, NUNITS * KU), np.float16)
        for iu, (ci, wdg, s) in enumerate(units):
            ks = np.arange(wdg * KW, wdg * KW + KU) % K
            d7all[:, iu * KU:(iu + 1) * KU] = np.stack(
                [dc[ks].astype(np.float16), ds[ks].astype(np.float16),
                 np.ones(KU, np.float16)])

        ins.append({
            "tris": tris_sel,
            "hh2": hh2,
            "hgt": hgt,
            "d7all": d7all,
        })
    return ins


_NC_CACHE = {}


def build_kernel():
    _key = "nc" + os.environ.get("K2_REPEAT", "1")
    if _key in _NC_CACHE:
        return _NC_CACHE[_key]
    import concourse.bacc as bacc
    import concourse.bass as bass
    import concourse.mybir as mybir
    from concourse.tile import TileContext, add_dep_helper

    dt = mybir.dt
    Alu = mybir.AluOpType
    Act = mybir.ActivationFunctionType
    AX = mybir.AxisListType.X
    IOff = bass.IndirectOffsetOnAxis

    nc = bacc.Bacc("TRN2", target_bir_lowering=False, debug=False,
                   num_devices=NCORES, num_swdge_queues=4)

    tris_d = nc.dram_tensor("tris", [2, NPART, FPP * 9], dt.float32,
                            kind="ExternalInput")
    hh2_d = nc.dram_tensor("hh2", [NPART, 3], dt.float32,
                           kind="ExternalInput")
    hgt_d = nc.dram_tensor("hgt", [4, 12], dt.float32, kind="ExternalInput")
    d7all_d = nc.dram_tensor("d7all", [3, NUNITS * KU], dt.float16,
                             kind="ExternalInput")
    out_d = nc.dram_tensor("out", [16], dt.float32, kind="ExternalOutput")

    with TileContext(nc) as tc:
        with (
            tc.tile_pool(name="const", bufs=1) as cp,
            tc.tile_pool(name="rep", bufs=2) as rpp,
            tc.tile_pool(name="slotbuf", bufs=2) as sp,
            tc.tile_pool(name="proj", bufs=2) as pp,
            tc.tile_pool(name="unit", bufs=3) as upl,
            tc.tile_pool(name="utail", bufs=4) as utp,
            tc.tile_pool(name="ref", bufs=4) as rp,
            tc.tile_pool(name="pst", bufs=1, space="PSUM") as pst,
            tc.tile_pool(name="psb", bufs=2, space="PSUM") as psb,
            tc.tile_pool(name="psn", bufs=2, space="PSUM") as psnp,
            tc.tile_pool(name="pss", bufs=2, space="PSUM") as pss,
            tc.tile_pool(name="dram", bufs=1, space="DRAM") as dmp,
            tc.tile_pool(name="dramu", bufs=1, space="DRAM") as dmu,
        ):
            ones = cp.tile([NPART, 1], dt.float32, tag="ones")
            nc.vector.memset(ones[:, :], 1.0)
            epseg = cp.tile([NPART, 1], dt.float32, tag="epseg")
            nc.vector.memset(epseg[:, :], 1e-20)
            outsb = cp.tile([1, 16], dt.float32, tag="outsb")
            nc.vector.memset(outsb[:, :], 0.0)
            pbase_u = cp.tile([NPART, 1], dt.uint32, tag="pbase_u")
            nc.gpsimd.iota(pbase_u[:, :], pattern=[[0, 1]], base=0,
                           channel_multiplier=PPP)
            pbase = cp.tile([NPART, 1], dt.float32, tag="pbase")
            nc.vector.tensor_copy(pbase[:, :], pbase_u[:, :])

            # identity (fp16) for the PE transpose
            onesb = cp.tile([NPART, NPART], dt.float16, tag="onesb")
            nc.vector.memset(onesb[:, :], 1.0)
            ident = cp.tile([NPART, NPART], dt.float16, tag="ident")
            nc.gpsimd.affine_select(ident[:, :], onesb[:, :],
                                    pattern=[[-1, NPART]], base=0,
                                    channel_multiplier=1,
                                    compare_op=Alu.is_equal, fill=0.0)
            # diff matrix D[p,i] = delta(p,i) - delta(p,i+1) (fp32)
            onesf = cp.tile([KU, KU], dt.float32, tag="onesf")
            nc.vector.memset(onesf[:, :], 1.0)
            dm1 = cp.tile([KU, KU], dt.float32, tag="dm1")
            nc.gpsimd.affine_select(dm1[:, :], onesf[:, :],
                                    pattern=[[-1, KU]], base=0,
                                    channel_multiplier=1,
                                    compare_op=Alu.is_equal, fill=0.0)
            dm2 = cp.tile([KU, KU], dt.float32, tag="dm2")
            nc.gpsimd.affine_select(dm2[:, :], onesf[:, :],
                                    pattern=[[-1, KU]], base=-1,
                                    channel_multiplier=1,
                                    compare_op=Alu.is_equal, fill=0.0)
            dmat = cp.tile([KU, KU], dt.float32, tag="dmat")
            nc.vector.tensor_sub(dmat[:, :], dm1[:, :], dm2[:, :])

            d7all = cp.tile([3, NUNITS * KU], dt.float16, tag="d7all")
            nc.sync.dma_start(d7all[:, :], d7all_d[:, :])

            scr = [dmp.tile([P, 3], dt.float32, tag=f"scr{s}{par}",
                            name=f"scr{s}{par}")
                   for s in range(2) for par in range(2)]
            prev_scr_readers = {}
            prev_svd_readers = {}
            _REPEAT = int(os.environ.get('K2_REPEAT', '1'))
            for _rep in range(_REPEAT):
                _par = _rep % 2
                hh2 = rpp.tile([NPART, 3], dt.float32, tag="hh2")
                nc.sync.dma_start(hh2[:, :], hh2_d[:, :])

                # ---- heights ----
                hgt = rpp.tile([4, 12], dt.float32, tag="hgt")
                nc.sync.dma_start(hgt[:, :], hgt_d[:, :])
                hp = rpp.tile([4, 6], dt.float32, tag="hp")
                nc.any.tensor_mul(hp[:, :], hgt[:, 0:6], hgt[:, 6:12])
                hs = rpp.tile([4, 1], dt.float32, tag="hs")
                nc.vector.reduce_sum(hs[:, :], hp[:, :], axis=AX)
                habs = rpp.tile([4, 1], dt.float32, tag="habs")
                nc.scalar.activation(habs[:, :], hs[:, :], Act.Abs)
                nc.sync.dma_start(out_d[11:15], habs[:, 0:1])

                for s in range(2):
                    slot_units = (list(range(6)) if s == 0
                                  else list(range(6, 9)))
                    T = sp.tile([NPART, FPP * 9], dt.float32, tag="tris")
                    tchunk = FPP * 9 // 4
                    for ch in range(4):
                        nc.sync.dma_start(
                            T[:, ch * tchunk:(ch + 1) * tchunk],
                            tris_d[s, :, ch * tchunk:(ch + 1) * tchunk])
                    Tv = T[:, :].rearrange("p (f n) -> p f n", n=9)

                    def cview(vtx, coord):
                        return Tv[:, :, 3 * vtx + coord]

                    # ---- mass: vol = z0*m12 + z1*m20 + z2*m01 ----
                    ta = sp.tile([NPART, FPP], dt.float32, tag="ta")
                    tb = sp.tile([NPART, FPP], dt.float32, tag="tb")
                    m12 = sp.tile([NPART, FPP], dt.float32, tag="m12")
                    m20 = sp.tile([NPART, FPP], dt.float32, tag="m20")
                    m01 = sp.tile([NPART, FPP], dt.float32, tag="m01")
                    for mt, (va, vb) in ((m12, (1, 2)), (m20, (2, 0)),
                                         (m01, (0, 1))):
                        nc.gpsimd.tensor_mul(ta[:, :], cview(va, 0),
                                             cview(vb, 1))
                        nc.gpsimd.tensor_mul(tb[:, :], cview(vb, 0),
                                             cview(va, 1))
                        nc.gpsimd.tensor_sub(mt[:, :], ta[:, :], tb[:, :])
                    macc = sp.tile([NPART, FPP], dt.float32, tag="macc")
                    mac2 = sp.tile([NPART, FPP], dt.float32, tag="mac2")
                    nc.gpsimd.tensor_mul(macc[:, :], cview(0, 2), m12[:, :])
                    nc.gpsimd.tensor_mul(mac2[:, :], cview(1, 2), m20[:, :])
                    nc.any.tensor_add(macc[:, :], macc[:, :], mac2[:, :])
                    nc.gpsimd.tensor_mul(mac2[:, :], cview(2, 2), m01[:, :])
                    nc.any.tensor_add(macc[:, :], macc[:, :], mac2[:, :])
                    msum = sp.tile([NPART, 1], dt.float32, tag="msum")
                    nc.vector.reduce_sum(msum[:, :], macc[:, :], axis=AX)
                    psm = pss.tile([1, 4], dt.float32, tag="small")
                    nc.tensor.matmul(psm[0:1, 0:1], lhsT=msum[:, :],
                                     rhs=ones[:, :], start=True, stop=True)
                    nc.scalar.activation(outsb[0:1, 9 + s:10 + s],
                                         psm[0:1, 0:1], Act.Abs,
                                         scale=DENSITY_OVER_6)

                    # ---- cross-section points (x, z, w) ----
                    pts3 = sp.tile([NPART, PPP * 3], dt.float32, tag="pts3")
                    p3v = pts3[:, :].rearrange("p (f e c) -> p f e c",
                                               e=3, c=3)
                    vmsk = sp.tile([NPART, PPP], dt.float32, tag="vmsk")
                    vv = vmsk[:, :].rearrange("p (f e) -> p f e", e=3)
                    hb = hh2[:, s:s + 1]
                    # u_v = h - y_v shared by both edges touching vertex v
                    uvt = []
                    for v in range(3):
                        uv = sp.tile([NPART, FPP], dt.float32, tag=f"uv{v}")
                        nc.scalar.activation(uv[:, :], cview(v, 1),
                                             Act.Identity, bias=hb,
                                             scale=-1.0)
                        uvt.append(uv)
                    for e in range(3):
                        i, j = e, (e + 1) % 3
                        yi = cview(i, 1); yj = cview(j, 1)
                        xi = cview(i, 0); xj = cview(j, 0)
                        zi = cview(i, 2); zj = cview(j, 2)
                        tnum = uvt[i]
                        tnum2 = uvt[j]
                        dd = sp.tile([NPART, FPP], dt.float32, tag="dd")
                        nc.gpsimd.tensor_sub(dd[:, :], yj, yi)
                        rec = sp.tile([NPART, FPP], dt.float32, tag="rec")
                        nc.vector.reciprocal(rec[:, :], dd[:, :])
                        trw = sp.tile([NPART, FPP], dt.float32, tag="trw")
                        nc.any.tensor_mul(trw[:, :], tnum[:, :], rec[:, :])
                        tcl = sp.tile([NPART, FPP], dt.float32, tag="tcl")
                        nc.any.tensor_scalar(tcl[:, :], trw[:, :], 0.0,
                                             1.0, op0=Alu.max, op1=Alu.min)
                        prod = sp.tile([NPART, FPP], dt.float32,
                                       tag="prodv")
                        nc.gpsimd.tensor_mul(prod[:, :], tnum[:, :],
                                             tnum2[:, :])
                        nc.any.tensor_scalar(vv[:, :, e], prod[:, :], 0.0,
                                             None, op0=Alu.is_lt)
                        nc.any.tensor_scalar(p3v[:, :, e, 2], vv[:, :, e],
                                             -NEG, NEG, op0=Alu.mult,
                                             op1=Alu.add)
                        dxt = sp.tile([NPART, FPP], dt.float32, tag="dxt")
                        nc.gpsimd.tensor_sub(dxt[:, :], xj, xi)
                        pxm = sp.tile([NPART, FPP], dt.float32, tag="pxm")
                        nc.gpsimd.tensor_mul(pxm[:, :], tcl[:, :], dxt[:, :])
                        nc.gpsimd.tensor_add(p3v[:, :, e, 0], pxm[:, :], xi)
                        dzt = sp.tile([NPART, FPP], dt.float32, tag="dzt")
                        nc.gpsimd.tensor_sub(dzt[:, :], zj, zi)
                        pzm = sp.tile([NPART, FPP], dt.float32, tag="pzm")
                        nc.gpsimd.tensor_mul(pzm[:, :], tcl[:, :], dzt[:, :])
                        nc.gpsimd.tensor_add(p3v[:, :, e, 1], pzm[:, :], zi)

                    pall = pts3[:, :].rearrange("p (n c) -> p n c", c=3)
                    xs = pall[:, :, 0]; zs = pall[:, :, 1]

                    # ---- centroid of valid points; center in place ----
                    cxt = sp.tile([NPART, PPP], dt.float32, tag="cxt")
                    s3 = sp.tile([NPART, 3], dt.float32, tag="s3")
                    nc.gpsimd.tensor_mul(cxt[:, :], xs, vmsk[:, :])
                    nc.vector.reduce_sum(s3[:, 0:1], cxt[:, :], axis=AX)
                    nc.gpsimd.tensor_mul(cxt[:, :], zs, vmsk[:, :])
                    nc.vector.reduce_sum(s3[:, 1:2], cxt[:, :], axis=AX)
                    cxt2 = sp.tile([NPART, PPP], dt.float32, tag="cxt2")
                    nc.scalar.activation(cxt2[:, :], vmsk[:, :],
                                         Act.Identity,
                                         accum_out=s3[:, 2:3])
                    ps3 = pss.tile([1, 4], dt.float32, tag="small")
                    nc.tensor.matmul(ps3[0:1, 0:3], lhsT=ones[:, :],
                                     rhs=s3[:, :], start=True, stop=True)
                    csum = sp.tile([1, 3], dt.float32, tag="csum")
                    nc.scalar.copy(csum[:, :], ps3[0:1, 0:3])
                    cneg = sp.tile([1, 1], dt.float32, tag="cneg")
                    nc.vector.tensor_scalar_mul(cneg[:, :], csum[0:1, 2:3],
                                                -1.0)
                    crec = sp.tile([1, 1], dt.float32, tag="crec")
                    nc.vector.reciprocal(crec[:, :], cneg[0:1, 0:1])
                    cxy = sp.tile([1, 2], dt.float32, tag="cxy")
                    nc.vector.tensor_scalar_mul(cxy[:, :], csum[0:1, 0:2],
                                                crec[0:1, 0:1])
                    cb = sp.tile([NPART, 2], dt.float32, tag="cb")
                    nc.gpsimd.partition_broadcast(cb[:, :], cxy[0:1, :],
                                                  NPART)
                    nc.gpsimd.tensor_scalar(xs, xs, cb[:, 0:1], None,
                                            op0=Alu.add)
                    nc.gpsimd.tensor_scalar(zs, zs, cb[:, 1:2], None,
                                            op0=Alu.add)

                    # centered fp32 copy to DRAM scratch (parity-buffered)
                    scr_s = scr[s * 2 + _par]
                    scr_view = scr_s[:, :].rearrange("(q n) c -> q (n c)",
                                                     q=NPART)
                    schunk = PPP * 3 // 4
                    w_scrs = []
                    for ch in range(4):
                        w = nc.sync.dma_start(
                            scr_view[:, ch * schunk:(ch + 1) * schunk],
                            pts3[:, ch * schunk:(ch + 1) * schunk])
                        for g in prev_scr_readers.get((s, _par), []):
                            add_dep_helper(w.ins, g,
                                           reason="scr WAR across reps")
                        w_scrs.append(w)
                    prev_scr_readers[(s, _par)] = []

                    # ---- bf16 copies (packed) for scoring ----
                    xb = pp.tile([NPART, PPP], dt.bfloat16, tag="xb")
                    nc.scalar.copy(xb[:, :], xs)
                    zb = pp.tile([NPART, PPP], dt.bfloat16, tag="zb")
                    nc.scalar.copy(zb[:, :], zs)
                    wbv = pp.tile([NPART, PPP], dt.bfloat16, tag="wbv")
                    nc.vector.tensor_copy(wbv[:, :], pall[:, :, 2])
                    if s == 1:
                        sgn = hh2[:, 2:3]
                        xq = pp.tile([NPART, PPP], dt.bfloat16, tag="xq")
                        nc.vector.tensor_scalar_mul(xq[:, :], xb[:, :], sgn)
                        zq = pp.tile([NPART, PPP], dt.bfloat16, tag="zq")
                        nc.vector.tensor_scalar_mul(zq[:, :], zb[:, :], sgn)
                        xb, zb = xq, zq

                    # shared boundary projections, w pre-added:
                    # a = x/2, b = z*s60; p60 = a+b, p120 = b-a
                    pa_ = pp.tile([NPART, PPP], dt.bfloat16, tag="pa_")
                    nc.vector.tensor_scalar_mul(pa_[:, :], xb[:, :], C60)
                    pbt = pp.tile([NPART, PPP], dt.bfloat16, tag="pbt")
                    nc.vector.tensor_scalar_mul(pbt[:, :], zb[:, :], S60)
                    p60 = pp.tile([NPART, PPP], dt.bfloat16, tag="p60")
                    nc.any.tensor_add(p60[:, :], pa_[:, :], pbt[:, :])
                    p120 = pp.tile([NPART, PPP], dt.bfloat16, tag="p120")
                    nc.any.tensor_sub(p120[:, :], pbt[:, :], pa_[:, :])
                    p0w = pp.tile([NPART, PPP], dt.bfloat16, tag="p0w")
                    nc.any.tensor_add(p0w[:, :], xb[:, :], wbv[:, :])
                    p60w = pp.tile([NPART, PPP], dt.bfloat16, tag="p60w")
                    nc.any.tensor_add(p60w[:, :], p60[:, :], wbv[:, :])
                    p120w = pp.tile([NPART, PPP], dt.bfloat16, tag="p120w")
                    nc.any.tensor_add(p120w[:, :], p120[:, :], wbv[:, :])
                    n0w = pp.tile([NPART, PPP], dt.bfloat16, tag="n0w")
                    nc.any.tensor_sub(n0w[:, :], wbv[:, :], xb[:, :])
                    if s == 0:
                        n60w = pp.tile([NPART, PPP], dt.bfloat16,
                                       tag="n60w")
                        nc.any.tensor_sub(n60w[:, :], wbv[:, :], p60[:, :])
                        n120w = pp.tile([NPART, PPP], dt.bfloat16,
                                        tag="n120w")
                        nc.any.tensor_sub(n120w[:, :], wbv[:, :],
                                          p120[:, :])
                        wpair = {0: (p0w, p60w), 1: (p60w, p120w),
                                 2: (p120w, n0w), 3: (n0w, n60w),
                                 4: (n60w, n120w), 5: (n120w, p0w)}
                    else:
                        wpair = {0: (p0w, p60w), 1: (p60w, p120w),
                                 2: (p120w, n0w)}

                    for u in slot_units:
                        wrel = u - 6 if s == 1 else u
                        pa, pb = wpair[wrel]
                        score = upl.tile([NPART, PPP], dt.bfloat16,
                                         tag="score")
                        nc.any.tensor_max(score[:, :], pa[:, :], pb[:, :])

                        # ---- top-2 per partition ----
                        mx8 = utp.tile([NPART, 8], dt.bfloat16, tag="mx8")
                        nc.vector.max(mx8[:, :], score[:, :])
                        i8 = utp.tile([NPART, 8], dt.uint16, tag="i8")
                        nc.vector.max_index(i8[:, :], mx8[:, :],
                                            score[:, :])
                        jf = utp.tile([NPART, SLOTS], dt.float32, tag="jf")
                        nc.vector.tensor_copy(jf[:, :], i8[:, 0:SLOTS])
                        gf = utp.tile([NPART, SLOTS], dt.float32, tag="gf")
                        nc.vector.tensor_scalar(gf[:, :], jf[:, :],
                                                pbase[:, 0:1], None,
                                                op0=Alu.add)
                        offs_u = utp.tile([NPART, SLOTS], dt.uint32,
                                          tag="offs_u")
                        nc.vector.tensor_copy(offs_u[:, :], gf[:, :])

                        # ---- survivor gathers ----
                        sg = utp.tile([NPART, SLOTS * 3], dt.float32,
                                      tag="sg")
                        for jslot in range(SLOTS):
                            g_sg = nc.gpsimd.indirect_dma_start(
                                out=sg[:, jslot * 3:jslot * 3 + 3],
                                out_offset=None,
                                in_=scr_s[:, :],
                                in_offset=IOff(
                                    ap=offs_u[:, jslot:jslot + 1], axis=0))
                            for w in w_scrs:
                                add_dep_helper(g_sg.ins, w.ins,
                                               reason="scr RAW")
                            prev_scr_readers[(s, _par)].append(g_sg.ins)

                        # block-major svd: row = slot*128 + partition
                        svd = dmu.tile([NSLOT, 3], dt.float32,
                                       tag=f"svd{u}{_par}",
                                       name=f"svd{u}{_par}")
                        w_svd = nc.sync.dma_start(
                            svd[:, :].rearrange("(n q) c -> q n c",
                                                q=NPART),
                            sg[:, :].rearrange("p (n c) -> p n c", c=3))
                        for g in prev_svd_readers.get((u, _par), []):
                            add_dep_helper(w_svd.ins, g,
                                           reason="svd WAR across reps")
                        prev_svd_readers[(u, _par)] = []

                        # ---- fp16 features [128, (slot,3)] ----
                        feat = utp.tile([NPART, SLOTS * 3], dt.float16,
                                        tag="feat")
                        nc.vector.tensor_copy(feat[:, :], sg[:, :])

                        # ---- PE transpose -> rhs [3, 256] fp16 ----
                        rhsb = utp.tile([3, NSLOT], dt.float16,
                                        tag="rhsb")
                        for sl in range(SLOTS):
                            psT = pst.tile([3, NPART], dt.float16,
                                           tag=f"psT{sl}")
                            nc.tensor.matmul(psT[:, :],
                                             lhsT=feat[:, sl * 3:sl * 3 + 3],
                                             rhs=ident[:, :],
                                             is_transpose=True,
                                             start=True, stop=True)
                            nc.vector.tensor_copy(
                                rhsb[:, sl * NPART:(sl + 1) * NPART],
                                psT[:, :])

                        # ---- survivor projections [121, 256] ----
                        ps2 = psb.tile([KU, NSLOT], dt.float32, tag="ps2")
                        for sl in range(SLOTS):
                            nc.tensor.matmul(
                                ps2[:, sl * NPART:(sl + 1) * NPART],
                                lhsT=d7all[:, u * KU:(u + 1) * KU],
                                rhs=rhsb[:, sl * NPART:(sl + 1) * NPART],
                                start=True, stop=True)

                        # ---- argmax over survivors ----
                        p8m = rp.tile([KU, 8], dt.float32, tag="p8m")
                        nc.vector.max(p8m[:, :], ps2[:, :])
                        i8b = rp.tile([KU, 8], dt.uint32, tag="i8b")
                        nc.vector.max_index(i8b[:, :], p8m[:, :],
                                            ps2[:, :])

                        # ---- extreme points; segments via diff matmul ----
                        ext = rp.tile([KU, 3], dt.float32, tag="ext")
                        g_ext = nc.gpsimd.indirect_dma_start(
                            out=ext[:, :], out_offset=None,
                            in_=svd[:, :],
                            in_offset=IOff(ap=i8b[:, 0:1], axis=0))
                        add_dep_helper(g_ext.ins, w_svd.ins,
                                       reason="svd RAW")
                        prev_svd_readers[(u, _par)].append(g_ext.ins)

                        psn = psnp.tile([KU, 2], dt.float32, tag="psn")
                        nc.tensor.matmul(psn[:, :], lhsT=dmat[:, :],
                                         rhs=ext[:, 0:2], start=True,
                                         stop=True)
                        sq = rp.tile([KW, 2], dt.float32, tag="sq")
                        nc.scalar.activation(sq[:, :], psn[0:KW, :],
                                             Act.Square)
                        ssum = rp.tile([KW, 1], dt.float32, tag="ssum")
                        nc.vector.reduce_sum(ssum[:, :], sq[:, :], axis=AX)
                        segl = rp.tile([KW, 1], dt.float32, tag="segl")
                        nc.scalar.activation(segl[:, :], ssum[:, :],
                                             Act.Sqrt,
                                             bias=epseg[0:KW, 0:1])
                        psq = pss.tile([1, 4], dt.float32, tag="small")
                        nc.tensor.matmul(psq[0:1, 0:1], lhsT=segl[:, :],
                                         rhs=ones[0:KW, :], start=True,
                                         stop=True)
                        nc.scalar.copy(outsb[0:1, u:u + 1],
                                       psq[0:1, 0:1])

                nc.sync.dma_start(out_d[0:11], outsb[0:1, 0:11])

    nc.compile()
    _NC_CACHE[_key] = nc
    return nc


def assemble(core_outs):
    perim = np.zeros(12, np.float64)
    for c in range(NCORES):
        for iu, (ci, wdg, s) in enumerate(SHARD[c]):
            perim[ci] += float(core_outs[c][iu])
    mass = np.array([core_outs[b][9] for b in range(4)], np.float32)
    height = np.asarray(core_outs[0][11:15], np.float32)
    out = np.stack([
        mass, height,
        perim[0:4].astype(np.float32),
        perim[4:8].astype(np.float32),
        perim[8:12].astype(np.float32),
    ])
    return out.astype(np.float32)


def kernel(triangles, head_top_bc, left_heel_bc, chest_bcs, belly_bcs,
           hips_bcs, head_top_face_idx, left_heel_face_idx,
           chest_face_index, belly_face_index, hips_face_index):
    from concourse import bass_utils

    faces = {"head": int(head_top_face_idx), "heel": int(left_heel_face_idx),
             "chest": int(chest_face_index), "belly": int(belly_face_index),
             "hips": int(hips_face_index)}
    bcs = {"head": np.asarray(head_top_bc, np.float32),
           "heel": np.asarray(left_heel_bc, np.float32),
           "chest": np.asarray(chest_bcs, np.float32),
           "belly": np.asarray(belly_bcs, np.float32),
           "hips": np.asarray(hips_bcs, np.float32)}
    tris = np.asarray(triangles, np.float32)

    ins = make_core_inputs(tris, faces, bcs)
    nc = build_kernel()
    res = bass_utils.run_bass_kernel_spmd(nc, ins,
                                          core_ids=list(range(NCORES)))
    return assemble([r["out"] for r in res.results])
